# revision 60
# baseline (speedup 1.0000x reference)
"""Trainium2 Bass kernel for nn_BinaryConnectNet (binary CNN, 8 NeuronCores).

Sharding: batch-parallel convs (128 img/core), fc1 output-feature-sharded
(128 features/core) with an on-device AllGather of the binary activations
(fp8, values +-1). fc2 computed as per-core partials, summed on host.

V2 structure (vs baseline):
 - conv1: dense K=82 matmul (81 = 27 taps x 3 bf16 splits of x, +1 ones row
   carrying the bias).  The 4 pool quadrants go to 4 psum banks; maxpool is
   computed BEFORE the sign: DVE f32 maxes psum pairs -> bf16, GPS maxes the
   bf16 pair, ACT applies Sign -> fp8 h1.  (max commutes with monotone
   rounding and sign, and bias is already inside the psum values.)
 - conv2 dw: fp8 DoubleRow diagonal matmuls, 2 taps per pass (5 passes for
   9 taps + zero pad), row-pipelined into the conv1 loop so the PE stays
   dense (p-state boost).  Integer-exact in fp8.
 - conv2 pw: fp8 matmul (values are small ints, exact); pool2 via the same
   max-then-sign eviction.
 - fc1: weights fp16 hi/lo preloaded into SBUF with large batched DMAs;
   rhs is the gathered +-1 activations in fp8.
 - fc2: fp16 hi/lo, per-core partial output in fp32.
"""

import sys

for _p in ("/opt/trn_rl_repo",):
    if _p not in sys.path:
        sys.path.insert(0, _p)

import numpy as np
import ml_dtypes
from contextlib import ExitStack

import concourse.bass as bass
import concourse.bacc as bacc
import concourse.mybir as mybir
import concourse.tile as tile
from concourse.ap import AP
from concourse.bass_utils import run_bass_kernel_spmd

F32 = mybir.dt.float32
BF16 = mybir.dt.bfloat16
FP16 = mybir.dt.float16
FP8 = mybir.dt.float8e4
AF = mybir.ActivationFunctionType
ALU = mybir.AluOpType
DR = mybir.MatmulPerfMode.DoubleRow

NCORES = 8
B = 128               # images per core
H = 32                # conv1 spatial
HP = 34               # padded
ROWLEN = B * HP       # 4352: one padded h-row across batch (b, w) flattened
X9_SLACK = 8
X9_ROW = HP * ROWLEN + X9_SLACK   # flattened (h, b, w) per (c, s) row + slack
P1 = 16               # pooled spatial after pool1
P1PAD = 18
P2 = 8                # pooled spatial after pool2
NF1 = 1024            # fc1 features (global)
FPC = NF1 // NCORES   # fc1 features per core = 128
KFC = 256 * P2 * P2   # fc1 contraction = 16384
NKT = KFC // 128      # 128 K-tiles
NB_ALL = NCORES * B   # 1024
DW_INTERLEAVE = True
IMT_BIG_DMA = True

# h1 is stored plane-major [128, y(19), b, x(XW=20)] so that a dw output
# row is one flat contiguous N run (all images side by side, 20-wide rows
# with 4 overscan columns whose outputs are discarded at eviction).
# DoubleRow k-subtile pairs must be DISJOINT in memory (overlapping pair
# strides crash the PE ifmap fetch), so taps are paired across du rows
# with a uniform delta of one plane; the du=2 taps pair with a zero slot
# that reads the (zeroed) extra plane 18.  lhsT slot order must match.
XW = 20
DW_PAIR_TAPS = [((0, 0), (1, 0)), ((0, 1), (1, 1)), ((0, 2), (1, 2)),
                ((2, 0), None), ((2, 1), None), ((2, 2), None)]


def _bf16(a):
    return np.asarray(a, dtype=ml_dtypes.bfloat16)


def _fp8(a):
    return np.asarray(a, dtype=ml_dtypes.float8_e4m3)


def _host_prep(x, w1_dw, b1_dw, w1_pw, b1_pw, w2_dw, b2_dw, w2_pw, b2_pw,
               fc1_w, fc1_b, fc2_w, fc2_b, ncores=NCORES, nb=B):
    """Build all per-core device input arrays (numpy only)."""
    sgn = np.sign
    x = np.asarray(x, np.float32)
    rowlen = nb * HP
    x9row = HP * rowlen + X9_SLACK
    fpc = FPC

    # triple bf16 split of x
    x0 = _bf16(x)
    r1 = x - x0.astype(np.float32)
    x1 = _bf16(r1)
    r2 = r1 - x1.astype(np.float32)
    x2 = _bf16(r2)
    splits = [x0, x1, x2]

    # x9h: per core [9 rows (3c+s) + ones row, x9row] bf16, (h, b, w), pad 1
    x9h = np.zeros((ncores, 10, x9row), dtype=ml_dtypes.bfloat16)
    for s in range(3):
        xs = splits[s].reshape(ncores, nb, 3, H, H)
        for c in range(3):
            row = np.zeros((ncores, HP, nb, HP), dtype=ml_dtypes.bfloat16)
            row[:, 1:33, :, 1:33] = xs[:, :, c].transpose(0, 2, 1, 3)
            x9h[:, 3 * c + s, : HP * rowlen] = row.reshape(ncores, -1)
    x9h[:, 9, :] = _bf16(1.0)

    # x81: fully host-side im2col [16 hc, 82 rows, 2*rowlen] so each conv1
    # row needs exactly one contiguous DMA (row 81 = ones for the bias row)
    x81 = np.empty((ncores, 16, 82, 2 * rowlen), dtype=ml_dtypes.bfloat16)
    for hc in range(16):
        for du in range(3):
            for dv in range(3):
                r0 = 9 * (3 * du + dv)
                off = (2 * hc + du) * rowlen + dv
                x81[:, hc, r0:r0 + 9, :] = x9h[:, 0:9,
                                               off:off + 2 * rowlen]
        x81[:, hc, 81, :] = _bf16(1.0)

    # conv1 fused weights + bias row: lhsT [82, 128]
    s1dw = sgn(np.asarray(w1_dw, np.float32))[:, 0]       # [3, 3, 3]
    s1pw = sgn(np.asarray(w1_pw, np.float32))[:, :, 0, 0]  # [128, 3]
    w1t = np.zeros((82, 128), dtype=ml_dtypes.bfloat16)
    for du in range(3):
        for dv in range(3):
            for c in range(3):
                for s in range(3):
                    w1t[9 * (3 * du + dv) + 3 * c + s] = _bf16(
                        s1pw[:, c] * s1dw[c, du, dv])
    b1eff = (sgn(np.asarray(b1_pw, np.float32))
             + s1pw @ sgn(np.asarray(b1_dw, np.float32))).astype(np.float32)
    w1t[81] = _bf16(b1eff)

    # conv2 depthwise: DoubleRow lhsT [128, 12, 128] fp8, slot order
    # matching DW_PAIR_TAPS (slot 2*i / 2*i+1 = pair i; None slot = zeros)
    s2dw = sgn(np.asarray(w2_dw, np.float32))[:, 0]       # [128, 3, 3]
    dwt = np.zeros((128, 12, 128), dtype=ml_dtypes.float8_e4m3)
    for i, (ta, tb) in enumerate(DW_PAIR_TAPS):
        np.fill_diagonal(dwt[:, 2 * i, :], _fp8(s2dw[:, ta[0], ta[1]]))
        if tb is not None:
            np.fill_diagonal(dwt[:, 2 * i + 1, :],
                             _fp8(s2dw[:, tb[0], tb[1]]))
    dwb = sgn(np.asarray(b2_dw, np.float32)).astype(np.float32)  # [128]

    # conv2 pointwise lhsT [128, 256] fp8 and bias
    s2pw = sgn(np.asarray(w2_pw, np.float32))[:, :, 0, 0]  # [256, 128]
    pwt = _fp8(s2pw.T)                                     # [128, 256]
    b2m = sgn(np.asarray(b2_pw, np.float32)).astype(np.float32)  # [256]

    # fc1 hi/lo fp16, column-permuted to device K-tile order, per-core slice
    fc1_w = np.asarray(fc1_w, np.float32)                  # [1024, 16384]
    cols = np.empty(KFC, np.int64)
    i = 0
    for ct in range(2):
        for s0 in range(64):
            for cp in range(128):
                cols[i] = (ct * 128 + cp) * 64 + s0
                i += 1
    wperm = fc1_w[:, cols]                                 # [1024, 16384(dev)]
    whi = wperm.astype(np.float16)
    wlo = (wperm - whi.astype(np.float32)).astype(np.float16)
    # per-core layout [128 (c' partition), NKT * 128 (kt, o)] so the SBUF
    # load is one fully-contiguous-per-partition DMA
    whi_t = whi.reshape(8, fpc, NKT, 128).transpose(0, 3, 2, 1).reshape(
        8, 128, NKT * fpc).copy()
    wlo_t = wlo.reshape(8, fpc, NKT, 128).transpose(0, 3, 2, 1).reshape(
        8, 128, NKT * fpc).copy()

    # fc2 hi/lo fp16 per-core slice: lhsT [128(f_local), 10]
    fc2_w = np.asarray(fc2_w, np.float32)                  # [10, 1024]
    f2 = fc2_w.T.reshape(8, fpc, 10)
    f2hi = f2.astype(np.float16)
    f2lo = (f2 - f2hi.astype(np.float32)).astype(np.float16)

    shared = {
        "w1t": w1t,
        "dwt": dwt.reshape(128, 12 * 128),
        "dwb": dwb.reshape(128, 1),
        "pwt": pwt,
        "b2m": b2m.reshape(2, 128).T.copy().astype(np.float32),
    }
    per_core = []
    for n in range(ncores):
        d = dict(shared)
        d["x81"] = x81[n].reshape(16 * 82, 2 * rowlen)
        d["whi"] = whi_t[n]
        d["wlo"] = wlo_t[n]
        d["f2hi"] = f2hi[n]
        d["f2lo"] = f2lo[n]
        per_core.append(d)
    return per_core


def _pair_ap(view, delta):
    """Insert an overlapping (delta, 2) dim at axis 1 of an AP view."""
    u = view.unsqueeze(1)
    dims = [list(d) for d in u.ap]
    dims[1] = [delta, 2]
    return AP(u.tensor, u.offset, dims)


def build_program(ncores=NCORES, nb=B):
    rowlen = nb * HP
    x9row = HP * rowlen + X9_SLACK
    nsh = ncores
    nball = ncores * nb
    bc_n = min(512, nball)
    nbc = nball // bc_n
    sh_per_bc = bc_n // nb

    nc = bacc.Bacc("TRN2", target_bir_lowering=False, debug=False,
                   num_devices=ncores)

    def din(name, shape, dt):
        return nc.dram_tensor(name, shape, dt, kind="ExternalInput").ap()

    x81 = din("x81", [16 * 82, 2 * rowlen], BF16)
    w1t = din("w1t", [82, 128], BF16)
    dwt = din("dwt", [128, 12 * 128], FP8)
    dwb = din("dwb", [128, 1], F32)
    pwt = din("pwt", [128, 256], FP8)
    b2m = din("b2m", [128, 2], F32)
    whi = din("whi", [128, NKT * FPC], FP16)
    wlo = din("wlo", [128, NKT * FPC], FP16)
    f2hi = din("f2hi", [FPC, 10], FP16)
    f2lo = din("f2lo", [FPC, 10], FP16)
    y_out = nc.dram_tensor("y", [10, nball], F32, kind="ExternalOutput").ap()

    h2_shard = nc.dram_tensor("h2_shard", [2, 128, nb * 64], FP8).ap()
    h2_all = nc.dram_tensor("h2_all", [nsh, 2, 128, nb * 64], FP8,
                            addr_space="Shared").ap()

    nh2 = H // 2   # 16 pooled rows after pool1
    NCH = 4        # conv1 b-chunks per row
    CB = nb // NCH  # images per chunk = 32

    with tile.TileContext(nc) as tc, ExitStack() as ctx:
        cpool = ctx.enter_context(tc.tile_pool(name="consts", bufs=1))
        w1_t = cpool.tile([82, 128], BF16)
        nc.sync.dma_start(w1_t[:], w1t[:])
        dw_t = cpool.tile([128, 12 * 128], FP8)
        nc.sync.dma_start(dw_t[:], dwt[:])
        dwv_t = dw_t[:].rearrange("p (t c) -> p t c", t=12)
        dwb_t = cpool.tile([128, 1], F32)
        nc.sync.dma_start(dwb_t[:], dwb[:])
        pw_t = cpool.tile([128, 256], FP8)
        nc.sync.dma_start(pw_t[:], pwt[:])
        b2_t = cpool.tile([128, 2], F32)
        nc.sync.dma_start(b2_t[:], b2m[:])

        # HAM warmup: back-to-back matmuls to ramp the PE clock while the
        # first imt DMAs land
        with tc.tile_pool(name="warm", bufs=1, space="PSUM") as wps:
            wp_t = wps.tile([128, 512], F32)
            for _w in range(16):
                nc.tensor.matmul(wp_t[:], dw_t[:, 0:128], dw_t[:, 0:512],
                                 start=(_w == 0), stop=(_w == 15))

        h2pool = ctx.enter_context(tc.tile_pool(name="h2", bufs=1))
        h2t = [h2pool.tile([128, nb * 64], FP8, tag=f"h2_{m}", name=f"h2_{m}")
               for m in range(2)]

        PLANE = nb * XW
        NPL = P1PAD + 1   # 18 data planes + 1 zero plane for the zero slots
        h1pool = ctx.enter_context(tc.tile_pool(name="h1p", bufs=1))
        h1p = h1pool.tile([128, NPL * PLANE + 64], FP8)
        h1v = h1p[:, 0:P1PAD * PLANE].rearrange(
            "p (y b x) -> p y b x", y=P1PAD, b=nb)
        # zero the pad borders, overscan columns, zero plane and tail
        nc.vector.memset(h1v[:, 0, :, :], 0.0)
        nc.vector.memset(h1v[:, P1PAD - 1, :, :], 0.0)
        nc.vector.memset(h1v[:, 1:P1PAD - 1, :, 0], 0.0)
        nc.vector.memset(h1v[:, 1:P1PAD - 1, :, 17:XW], 0.0)
        nc.vector.memset(h1p[:, P1PAD * PLANE:], 0.0)

        dwcpool = ctx.enter_context(tc.tile_pool(name="dwc", bufs=1))
        dwc = dwcpool.tile([128, nb * 256 + 16], FP8)
        # dwc layout: (b, y, x) with y,x in 16x16
        dwcv = dwc[:, 0:nb * 256].rearrange("p (b y x) -> p b y x",
                                            b=nb, y=P1)

        # DMA issue rotation, weighted toward engines with idle capacity
        # (only sync/gpsimd/scalar can trigger DMAs)
        dma_engines = [nc.sync, nc.gpsimd, nc.sync, nc.gpsimd,
                       nc.sync, nc.gpsimd, nc.sync, nc.scalar]

        DW_CHUNKS = [(0, 25), (25, 25), (50, 25), (75, 25), (100, 25),
                     (125, 3)]

        def dw_row(yo, b0, cnt, dps):
            """conv2 depthwise for output row yo (0..15), images b0..b0+cnt.

            Reads padded h1 planes yo..yo+2; plane yo+2 is written by conv1
            pooled row hc=yo+1 (or is the bottom pad for yo=15).  The rhs is
            a flat [p, 2, cnt*XW] DoubleRow AP; overscan columns (x>=16)
            are discarded at eviction.
            """
            pd = dps.tile([128, 25 * XW], F32, tag="dps")
            n = cnt * XW
            npair = len(DW_PAIR_TAPS)
            for i, (ta, tb) in enumerate(DW_PAIR_TAPS):
                offs = (yo + ta[0]) * PLANE + b0 * XW + ta[1]
                if tb is not None:
                    delta = (tb[0] - ta[0]) * PLANE + (tb[1] - ta[1])
                else:
                    delta = 0   # zero slot re-reads slot 0 (always written)
                dims = [list(h1p[:].ap[0]), [delta, 2], [1, n]]
                rhs = AP(h1p[:].tensor, offs, dims)
                nc.tensor.matmul(pd[:, 0:n], dwv_t[:, 2 * i:2 * i + 2, :],
                                 rhs, start=(i == 0), stop=(i == npair - 1),
                                 perf_mode=DR)
            nc.scalar.activation(
                dwcv[:, b0:b0 + cnt, yo, :],
                pd[:, 0:n].rearrange("p (b x) -> p b x", b=cnt)[:, :, 0:P1],
                AF.Identity, bias=dwb_t[:])

        with tc.tile_pool(name="c1im", bufs=2) as impool, \
             tc.tile_pool(name="c1ev", bufs=2) as evpool, \
             tc.tile_pool(name="c1ps", bufs=1, space="PSUM") as pspool, \
             tc.tile_pool(name="dwps", bufs=2, space="PSUM") as dps:
            for hc in range(nh2):
                imt = impool.tile([82, 2 * rowlen], BF16, tag="im")
                # split across both HWDGE rings (sync + scalar) + SWDGE
                nc.sync.dma_start(imt[0:28, :],
                                  x81[hc * 82:hc * 82 + 28, :])
                nc.scalar.dma_start(imt[28:56, :],
                                    x81[hc * 82 + 28:hc * 82 + 56, :])
                nc.gpsimd.dma_start(imt[56:82, :],
                                    x81[hc * 82 + 56:hc * 82 + 82, :])
                imv = imt[:].rearrange("p (h b w2 dx) -> p h b w2 dx",
                                       h=2, b=nb, w2=HP // 2)
                for c in range(NCH):
                    qs = []
                    for dy in range(2):
                        for dx in range(2):
                            ps = pspool.tile([128, CB * 16], F32,
                                             tag=f"q{2 * dy + dx}")
                            nc.tensor.matmul(
                                ps[:], w1_t[:],
                                imv[:, dy, c * CB:(c + 1) * CB, 0:16, dx],
                                start=True, stop=True)
                            qs.append(ps)
                    out_sl = h1v[:, hc + 1, c * CB:(c + 1) * CB, 1:17]
                    # max-then-sign eviction; sign commutes with max, and
                    # TT may read at most one PSUM operand.  Alternate two
                    # chain types to balance ACT vs DVE load.
                    if (hc * NCH + c) % 8 < 3:
                        # X: ACT stages 2 raw copies, DVE maxes, ACT signs
                        e0 = evpool.tile([128, CB * 16], F32, tag="e0")
                        e2 = evpool.tile([128, CB * 16], F32, tag="e2")
                        nc.scalar.copy(e0[:], qs[0][:])
                        nc.scalar.copy(e2[:], qs[2][:])
                        t01 = evpool.tile([128, CB * 16], BF16, tag="t01")
                        t23 = evpool.tile([128, CB * 16], BF16, tag="t23")
                        nc.vector.tensor_max(t01[:], qs[1][:], e0[:])
                        nc.vector.tensor_max(t23[:], qs[3][:], e2[:])
                        tm = evpool.tile([128, CB * 16], BF16, tag="tm")
                        nc.vector.tensor_max(tm[:], t01[:], t23[:])
                        nc.scalar.activation(
                            out_sl, tm[:].rearrange("p (b x) -> p b x", b=CB),
                            AF.Sign)
                    else:
                        # Y: ACT signs all 4 quadrants, DVE bf16 max tree
                        ss = []
                        for q in range(4):
                            s = evpool.tile([128, CB * 16], BF16, tag=f"s{q}")
                            nc.scalar.activation(s[:], qs[q][:], AF.Sign)
                            ss.append(s)
                        y01 = evpool.tile([128, CB * 16], BF16, tag="t01")
                        y23 = evpool.tile([128, CB * 16], BF16, tag="t23")
                        nc.vector.tensor_max(y01[:], ss[0][:], ss[1][:])
                        nc.vector.tensor_max(y23[:], ss[2][:], ss[3][:])
                        nc.vector.tensor_max(
                            out_sl,
                            y01[:].rearrange("p (b x) -> p b x", b=CB),
                            y23[:].rearrange("p (b x) -> p b x", b=CB))
                    if DW_INTERLEAVE and hc >= 1:
                        # dw chunks interleave between conv1 chunks: keeps
                        # the PE dense while DVE/ACT drain the quad psums.
                        # chunk c of DW_CHUNKS covers images <= conv1 chunk c
                        for b0, cnt in DW_CHUNKS[c:c + (3 if c == 3 else 1)]:
                            dw_row(hc - 1, b0, cnt, dps)
            if DW_INTERLEAVE:
                for b0, cnt in DW_CHUNKS:
                    dw_row(nh2 - 1, b0, cnt, dps)   # last row, bottom pad
            else:
                for yo in range(nh2):
                    for b0, cnt in DW_CHUNKS:
                        dw_row(yo, b0, cnt, dps)

        # ---- conv2 pointwise + pool2 -> h2t (fp8 +-1) ----
        dwc6 = dwc[:, 0:nb * 256].rearrange(
            "p (b y2 dy x2 dx) -> p b y2 dy x2 dx", b=nb, y2=P2, dy=2, x2=P2)
        with tc.tile_pool(name="pwev", bufs=2) as pevpool, \
             tc.tile_pool(name="pwps", bufs=2, space="PSUM") as pps:
            PJ = 8   # images per pw chunk
            for mt in range(2):
                for j in range(nb // PJ):
                    b0 = j * PJ
                    qs = []
                    for dy in range(2):
                        for dx in range(2):
                            ps = pps.tile([128, PJ * 64], F32,
                                          tag=f"pq{2 * dy + dx}")
                            rhs = dwc6[:, b0:b0 + PJ, :, dy, :, dx]
                            nc.tensor.matmul(
                                ps[:], pw_t[:, 128 * mt:128 * (mt + 1)],
                                rhs, start=True, stop=True)
                            qs.append(ps)
                    h2vv = h2t[mt][:].rearrange("p (s b) -> p b s", b=nb)
                    out_sl = h2vv[:, b0:b0 + PJ, :]
                    if True:
                        g0 = pevpool.tile([128, PJ * 64], F32, tag="g0")
                        g2 = pevpool.tile([128, PJ * 64], F32, tag="g2")
                        nc.scalar.copy(g0[:], qs[0][:])
                        nc.scalar.copy(g2[:], qs[2][:])
                        u01 = pevpool.tile([128, PJ * 64], BF16, tag="u01")
                        u23 = pevpool.tile([128, PJ * 64], BF16, tag="u23")
                        nc.vector.tensor_max(u01[:], qs[1][:], g0[:])
                        nc.vector.tensor_max(u23[:], qs[3][:], g2[:])
                        um = pevpool.tile([128, PJ * 64], BF16, tag="um")
                        nc.vector.tensor_max(um[:], u01[:], u23[:])
                        nc.scalar.activation(
                            out_sl, um[:].rearrange("p (b s) -> p b s", b=PJ),
                            AF.Sign, bias=b2_t[:, mt:mt + 1])
                    else:
                        ss = []
                        for q in range(4):
                            s = pevpool.tile([128, PJ * 64], BF16,
                                             tag=f"ps{q}")
                            nc.scalar.activation(s[:], qs[q][:], AF.Sign,
                                                 bias=b2_t[:, mt:mt + 1])
                            ss.append(s)
                        v01 = pevpool.tile([128, PJ * 64], BF16, tag="u01")
                        v23 = pevpool.tile([128, PJ * 64], BF16, tag="u23")
                        nc.vector.tensor_max(v01[:], ss[0][:], ss[1][:])
                        nc.vector.tensor_max(v23[:], ss[2][:], ss[3][:])
                        nc.vector.tensor_max(
                            out_sl,
                            v01[:].rearrange("p (b s) -> p b s", b=PJ),
                            v23[:].rearrange("p (b s) -> p b s", b=PJ))
        for mt in range(2):
            nc.sync.dma_start(h2_shard[mt], h2t[mt][:])

        import os as _os
        if _os.environ.get("BCN_DEBUG"):
            dbg_h1 = nc.dram_tensor("dbg_h1", [128, NPL * nb * XW + 64],
                                    FP8, kind="ExternalOutput").ap()
            nc.sync.dma_start(dbg_h1[:], h1p[:])
            dbg_dwc = nc.dram_tensor("dbg_dwc", [128, nb * 256 + 16], FP8,
                                     kind="ExternalOutput").ap()
            nc.sync.dma_start(dbg_dwc[:], dwc[:])
            dbg_h2 = nc.dram_tensor("dbg_h2", [2, 128, nb * 64], FP8,
                                    kind="ExternalOutput").ap()
            for mt in range(2):
                nc.sync.dma_start(dbg_h2[mt], h2t[mt][:])

    # ---- AllGather (raw phase) ----
    # No all_engine_barrier afterwards: all reads of h2_all in the fc phase
    # are issued from the gpsimd queue, which program-orders them after the
    # collective wait below; other engines (fc weight loads) may proceed
    # during the collective.
    if ncores > 1:
        with nc.Block() as blk, nc.semaphore("cc_sem") as cc_sem:
            @blk.gpsimd
            def _(gp):
                gp.collective_compute(
                    "AllGather", ALU.bypass,
                    replica_groups=[list(range(ncores))],
                    ins=[h2_shard], outs=[h2_all],
                ).then_inc(cc_sem)
                gp.wait_ge(cc_sem, 1)
        nc.all_engine_barrier()
    else:
        with nc.Block() as blk, nc.semaphore("cp_sem") as cp_sem:
            @blk.gpsimd
            def _(gp):
                gp.dma_start(h2_all[0], h2_shard[:]).then_inc(cp_sem, 16)
                gp.wait_ge(cp_sem, 16)
        nc.all_engine_barrier()

    # ---- fc1 + fc2 (Tile phase 2) ----
    with tile.TileContext(nc) as tc2, ExitStack() as ctx2:
        hgp = ctx2.enter_context(tc2.tile_pool(name="hg", bufs=1))
        wp = ctx2.enter_context(tc2.tile_pool(name="wfc", bufs=1))
        sp = ctx2.enter_context(tc2.tile_pool(name="fc1out", bufs=1))
        psp = ctx2.enter_context(tc2.tile_pool(name="fcps", bufs=1,
                                               space="PSUM"))
        p10 = ctx2.enter_context(tc2.tile_pool(name="fc2ps", bufs=2,
                                               space="PSUM"))
        yp = ctx2.enter_context(tc2.tile_pool(name="yout", bufs=1))
        dma_engines2 = [nc.sync, nc.gpsimd, nc.scalar]

        whi_sb = wp.tile([128, NKT * FPC], FP16, tag="whi", name="whi_sb")
        wlo_sb = wp.tile([128, NKT * FPC], FP16, tag="wlo", name="wlo_sb")
        whi_v = whi_sb[:].rearrange("p (k c) -> p k c", k=NKT)
        wlo_v = wlo_sb[:].rearrange("p (k c) -> p k c", k=NKT)

        hg = {}
        for ct in range(2):
            for bcc in range(nbc):
                t = hgp.tile([128, sh_per_bc * nb * 64], FP8,
                             tag=f"hg{ct}{bcc}", name=f"hg{ct}{bcc}")
                hg[(ct, bcc)] = t

        # weight loads (no collective dependence) overlap the AllGather;
        # hg loads MUST be issued from gpsimd (queued after the cc wait)
        HKT = NKT * FPC // 2
        nc.sync.dma_start(whi_sb[:, 0:HKT], whi[:, 0:HKT])
        nc.scalar.dma_start(wlo_sb[:, 0:HKT], wlo[:, 0:HKT])
        nc.sync.dma_start(whi_sb[:, HKT:], whi[:, HKT:])
        nc.scalar.dma_start(wlo_sb[:, HKT:], wlo[:, HKT:])
        SHB = nb * 64
        for bcc in range(nbc):
            for ct in range(2):
                src = AP(h2_all.tensor,
                         (bcc * sh_per_bc * 2 + ct) * 128 * SHB,
                         [[SHB, 128], [2 * 128 * SHB, sh_per_bc], [1, SHB]])
                nc.gpsimd.dma_start(hg[(ct, bcc)][:], src)

        s1 = sp.tile([128, nball], FP8)
        psf = [psp.tile([128, bc_n], F32, tag=f"psf{bcc}", name=f"psf{bcc}")
               for bcc in range(nbc)]
        f2hi_t = sp.tile([128, 10], FP16)
        nc.sync.dma_start(f2hi_t[:], f2hi[:])
        f2lo_t = sp.tile([128, 10], FP16)
        nc.sync.dma_start(f2lo_t[:], f2lo[:])

        yt = yp.tile([10, nball], F32)
        for bcc in range(nbc):
            for kt in range(NKT):
                ct, s0 = kt // 64, kt % 64
                rhs = hg[(ct, bcc)][:].rearrange(
                    "p (s x b) -> p s x b",
                    s=sh_per_bc, b=nb)[:, :, s0, :]
                nc.tensor.matmul(psf[bcc][:], whi_v[:, kt, :], rhs,
                                 start=(kt == 0), stop=False)
                nc.tensor.matmul(psf[bcc][:], wlo_v[:, kt, :], rhs,
                                 start=False, stop=(kt == NKT - 1))
            nc.scalar.activation(s1[:, bcc * bc_n:(bcc + 1) * bc_n],
                                 psf[bcc][:], AF.Sign)
            ps10 = p10.tile([10, bc_n], F32, tag="ps10")
            nc.tensor.matmul(ps10[:], f2hi_t[:],
                             s1[:, bcc * bc_n:(bcc + 1) * bc_n],
                             start=True, stop=False)
            nc.tensor.matmul(ps10[:], f2lo_t[:],
                             s1[:, bcc * bc_n:(bcc + 1) * bc_n],
                             start=False, stop=True)
            nc.scalar.copy(yt[:, bcc * bc_n:(bcc + 1) * bc_n], ps10[:])
        nc.sync.dma_start(y_out[:], yt[:])

    nc.compile()
    return nc


_CACHE = {}


def _get_program(ncores=NCORES, nb=B):
    key = (ncores, nb)
    if key not in _CACHE:
        _CACHE[key] = build_program(ncores, nb)
    return _CACHE[key]


def kernel(**inputs):
    per_core = _host_prep(**inputs)
    nc = _get_program()
    res = run_bass_kernel_spmd(nc, per_core, core_ids=list(range(NCORES)))
    fc2_b = np.asarray(inputs["fc2_b"], np.float32)
    y = np.zeros((10, NB_ALL), np.float32)
    for n in range(NCORES):
        y += res.results[n]["y"]
    return (y.T + fc2_b[None, :]).astype(np.float32)


# revision 61
# speedup vs baseline: 1.0057x; 1.0057x over previous
"""Trainium2 Bass kernel for nn_BinaryConnectNet (binary CNN, 8 NeuronCores).

Sharding: batch-parallel convs (128 img/core), fc1 output-feature-sharded
(128 features/core) with an on-device AllGather of the binary activations
(fp8, values +-1). fc2 computed as per-core partials, summed on host.

V2 structure (vs baseline):
 - conv1: dense K=82 matmul (81 = 27 taps x 3 bf16 splits of x, +1 ones row
   carrying the bias).  The 4 pool quadrants go to 4 psum banks; maxpool is
   computed BEFORE the sign: DVE f32 maxes psum pairs -> bf16, GPS maxes the
   bf16 pair, ACT applies Sign -> fp8 h1.  (max commutes with monotone
   rounding and sign, and bias is already inside the psum values.)
 - conv2 dw: fp8 DoubleRow diagonal matmuls, 2 taps per pass (5 passes for
   9 taps + zero pad), row-pipelined into the conv1 loop so the PE stays
   dense (p-state boost).  Integer-exact in fp8.
 - conv2 pw: fp8 matmul (values are small ints, exact); pool2 via the same
   max-then-sign eviction.
 - fc1: weights fp16 hi/lo preloaded into SBUF with large batched DMAs;
   rhs is the gathered +-1 activations in fp8.
 - fc2: fp16 hi/lo, per-core partial output in fp32.
"""

import sys

for _p in ("/opt/trn_rl_repo",):
    if _p not in sys.path:
        sys.path.insert(0, _p)

import numpy as np
import ml_dtypes
from contextlib import ExitStack

import concourse.bass as bass
import concourse.bacc as bacc
import concourse.mybir as mybir
import concourse.tile as tile
from concourse.ap import AP
from concourse.bass_utils import run_bass_kernel_spmd

F32 = mybir.dt.float32
BF16 = mybir.dt.bfloat16
FP16 = mybir.dt.float16
FP8 = mybir.dt.float8e4
AF = mybir.ActivationFunctionType
ALU = mybir.AluOpType
DR = mybir.MatmulPerfMode.DoubleRow

NCORES = 8
B = 128               # images per core
H = 32                # conv1 spatial
HP = 34               # padded
ROWLEN = B * HP       # 4352: one padded h-row across batch (b, w) flattened
X9_SLACK = 8
X9_ROW = HP * ROWLEN + X9_SLACK   # flattened (h, b, w) per (c, s) row + slack
P1 = 16               # pooled spatial after pool1
P1PAD = 18
P2 = 8                # pooled spatial after pool2
NF1 = 1024            # fc1 features (global)
FPC = NF1 // NCORES   # fc1 features per core = 128
KFC = 256 * P2 * P2   # fc1 contraction = 16384
NKT = KFC // 128      # 128 K-tiles
NB_ALL = NCORES * B   # 1024
DW_INTERLEAVE = True
IMT_BIG_DMA = True

# h1 is stored plane-major [128, y(19), b, x(XW=20)] so that a dw output
# row is one flat contiguous N run (all images side by side, 20-wide rows
# with 4 overscan columns whose outputs are discarded at eviction).
# DoubleRow k-subtile pairs must be DISJOINT in memory (overlapping pair
# strides crash the PE ifmap fetch), so taps are paired across du rows
# with a uniform delta of one plane; the du=2 taps pair with a zero slot
# that reads the (zeroed) extra plane 18.  lhsT slot order must match.
XW = 20
DW_PAIR_TAPS = [((0, 0), (1, 0)), ((0, 1), (1, 1)), ((0, 2), (1, 2)),
                ((2, 0), None), ((2, 1), None), ((2, 2), None)]


def _bf16(a):
    return np.asarray(a, dtype=ml_dtypes.bfloat16)


def _fp8(a):
    return np.asarray(a, dtype=ml_dtypes.float8_e4m3)


def _host_prep(x, w1_dw, b1_dw, w1_pw, b1_pw, w2_dw, b2_dw, w2_pw, b2_pw,
               fc1_w, fc1_b, fc2_w, fc2_b, ncores=NCORES, nb=B):
    """Build all per-core device input arrays (numpy only)."""
    sgn = np.sign
    x = np.asarray(x, np.float32)
    rowlen = nb * HP
    x9row = HP * rowlen + X9_SLACK
    fpc = FPC

    # triple bf16 split of x
    x0 = _bf16(x)
    r1 = x - x0.astype(np.float32)
    x1 = _bf16(r1)
    r2 = r1 - x1.astype(np.float32)
    x2 = _bf16(r2)
    splits = [x0, x1, x2]

    # x9h: per core [9 rows (3c+s) + ones row, x9row] bf16, (h, b, w), pad 1
    x9h = np.zeros((ncores, 10, x9row), dtype=ml_dtypes.bfloat16)
    for s in range(3):
        xs = splits[s].reshape(ncores, nb, 3, H, H)
        for c in range(3):
            row = np.zeros((ncores, HP, nb, HP), dtype=ml_dtypes.bfloat16)
            row[:, 1:33, :, 1:33] = xs[:, :, c].transpose(0, 2, 1, 3)
            x9h[:, 3 * c + s, : HP * rowlen] = row.reshape(ncores, -1)
    x9h[:, 9, :] = _bf16(1.0)

    # x81: fully host-side im2col [16 hc, 82 rows, 2*rowlen] so each conv1
    # row needs exactly one contiguous DMA (row 81 = ones for the bias row)
    x81 = np.empty((ncores, 16, 82, 2 * rowlen), dtype=ml_dtypes.bfloat16)
    for hc in range(16):
        for du in range(3):
            for dv in range(3):
                r0 = 9 * (3 * du + dv)
                off = (2 * hc + du) * rowlen + dv
                x81[:, hc, r0:r0 + 9, :] = x9h[:, 0:9,
                                               off:off + 2 * rowlen]
        x81[:, hc, 81, :] = _bf16(1.0)

    # conv1 fused weights + bias row: lhsT [82, 128]
    s1dw = sgn(np.asarray(w1_dw, np.float32))[:, 0]       # [3, 3, 3]
    s1pw = sgn(np.asarray(w1_pw, np.float32))[:, :, 0, 0]  # [128, 3]
    w1t = np.zeros((82, 128), dtype=ml_dtypes.bfloat16)
    for du in range(3):
        for dv in range(3):
            for c in range(3):
                for s in range(3):
                    w1t[9 * (3 * du + dv) + 3 * c + s] = _bf16(
                        s1pw[:, c] * s1dw[c, du, dv])
    b1eff = (sgn(np.asarray(b1_pw, np.float32))
             + s1pw @ sgn(np.asarray(b1_dw, np.float32))).astype(np.float32)
    w1t[81] = _bf16(b1eff)

    # conv2 depthwise: DoubleRow lhsT [128, 12, 128] fp8, slot order
    # matching DW_PAIR_TAPS (slot 2*i / 2*i+1 = pair i; None slot = zeros)
    s2dw = sgn(np.asarray(w2_dw, np.float32))[:, 0]       # [128, 3, 3]
    dwt = np.zeros((128, 12, 128), dtype=ml_dtypes.float8_e4m3)
    for i, (ta, tb) in enumerate(DW_PAIR_TAPS):
        np.fill_diagonal(dwt[:, 2 * i, :], _fp8(s2dw[:, ta[0], ta[1]]))
        if tb is not None:
            np.fill_diagonal(dwt[:, 2 * i + 1, :],
                             _fp8(s2dw[:, tb[0], tb[1]]))
    dwb = sgn(np.asarray(b2_dw, np.float32)).astype(np.float32)  # [128]

    # conv2 pointwise lhsT [128, 256] fp8 and bias
    s2pw = sgn(np.asarray(w2_pw, np.float32))[:, :, 0, 0]  # [256, 128]
    pwt = _fp8(s2pw.T)                                     # [128, 256]
    b2m = sgn(np.asarray(b2_pw, np.float32)).astype(np.float32)  # [256]

    # fc1 hi/lo fp16, column-permuted to device K-tile order, per-core slice
    fc1_w = np.asarray(fc1_w, np.float32)                  # [1024, 16384]
    cols = np.empty(KFC, np.int64)
    i = 0
    for ct in range(2):
        for s0 in range(64):
            for cp in range(128):
                cols[i] = (ct * 128 + cp) * 64 + s0
                i += 1
    wperm = fc1_w[:, cols]                                 # [1024, 16384(dev)]
    whi = wperm.astype(np.float16)
    wlo = (wperm - whi.astype(np.float32)).astype(np.float16)
    # per-core layout [128 (c' partition), NKT * 128 (kt, o)] so the SBUF
    # load is one fully-contiguous-per-partition DMA
    whi_t = whi.reshape(8, fpc, NKT, 128).transpose(0, 3, 2, 1).reshape(
        8, 128, NKT * fpc).copy()
    wlo_t = wlo.reshape(8, fpc, NKT, 128).transpose(0, 3, 2, 1).reshape(
        8, 128, NKT * fpc).copy()

    # fc2 hi/lo fp16 per-core slice: lhsT [128(f_local), 10]
    fc2_w = np.asarray(fc2_w, np.float32)                  # [10, 1024]
    f2 = fc2_w.T.reshape(8, fpc, 10)
    f2hi = f2.astype(np.float16)
    f2lo = (f2 - f2hi.astype(np.float32)).astype(np.float16)

    shared = {
        "w1t": w1t,
        "dwt": dwt.reshape(128, 12 * 128),
        "dwb": dwb.reshape(128, 1),
        "pwt": pwt,
        "b2m": b2m.reshape(2, 128).T.copy().astype(np.float32),
    }
    per_core = []
    for n in range(ncores):
        d = dict(shared)
        d["x81"] = x81[n].reshape(16 * 82, 2 * rowlen)
        d["whi"] = whi_t[n]
        d["wlo"] = wlo_t[n]
        d["f2hi"] = f2hi[n]
        d["f2lo"] = f2lo[n]
        per_core.append(d)
    return per_core


def _pair_ap(view, delta):
    """Insert an overlapping (delta, 2) dim at axis 1 of an AP view."""
    u = view.unsqueeze(1)
    dims = [list(d) for d in u.ap]
    dims[1] = [delta, 2]
    return AP(u.tensor, u.offset, dims)


def build_program(ncores=NCORES, nb=B):
    rowlen = nb * HP
    x9row = HP * rowlen + X9_SLACK
    nsh = ncores
    nball = ncores * nb
    bc_n = min(512, nball)
    nbc = nball // bc_n
    sh_per_bc = bc_n // nb

    nc = bacc.Bacc("TRN2", target_bir_lowering=False, debug=False,
                   num_devices=ncores)

    def din(name, shape, dt):
        return nc.dram_tensor(name, shape, dt, kind="ExternalInput").ap()

    x81 = din("x81", [16 * 82, 2 * rowlen], BF16)
    w1t = din("w1t", [82, 128], BF16)
    dwt = din("dwt", [128, 12 * 128], FP8)
    dwb = din("dwb", [128, 1], F32)
    pwt = din("pwt", [128, 256], FP8)
    b2m = din("b2m", [128, 2], F32)
    whi = din("whi", [128, NKT * FPC], FP16)
    wlo = din("wlo", [128, NKT * FPC], FP16)
    f2hi = din("f2hi", [FPC, 10], FP16)
    f2lo = din("f2lo", [FPC, 10], FP16)
    y_out = nc.dram_tensor("y", [10, nball], F32, kind="ExternalOutput").ap()

    h2_shard = nc.dram_tensor("h2_shard", [2, 128, nb * 64], FP8).ap()
    h2_all = nc.dram_tensor("h2_all", [nsh, 2, 128, nb * 64], FP8,
                            addr_space="Shared").ap()

    nh2 = H // 2   # 16 pooled rows after pool1
    NCH = 4        # conv1 b-chunks per row
    CB = nb // NCH  # images per chunk = 32

    with tile.TileContext(nc) as tc, ExitStack() as ctx:
        cpool = ctx.enter_context(tc.tile_pool(name="consts", bufs=1))
        w1_t = cpool.tile([82, 128], BF16)
        nc.sync.dma_start(w1_t[:], w1t[:])
        dw_t = cpool.tile([128, 12 * 128], FP8)
        nc.sync.dma_start(dw_t[:], dwt[:])
        dwv_t = dw_t[:].rearrange("p (t c) -> p t c", t=12)
        dwb_t = cpool.tile([128, 1], F32)
        nc.sync.dma_start(dwb_t[:], dwb[:])
        pw_t = cpool.tile([128, 256], FP8)
        nc.sync.dma_start(pw_t[:], pwt[:])
        b2_t = cpool.tile([128, 2], F32)
        nc.sync.dma_start(b2_t[:], b2m[:])

        # HAM warmup: back-to-back matmuls to ramp the PE clock while the
        # first imt DMAs land
        with tc.tile_pool(name="warm", bufs=1, space="PSUM") as wps:
            wp_t = wps.tile([128, 512], F32)
            for _w in range(16):
                nc.tensor.matmul(wp_t[:], dw_t[:, 0:128], dw_t[:, 0:512],
                                 start=(_w == 0), stop=(_w == 15))

        h2pool = ctx.enter_context(tc.tile_pool(name="h2", bufs=1))
        h2t = [h2pool.tile([128, nb * 64], FP8, tag=f"h2_{m}", name=f"h2_{m}")
               for m in range(2)]

        PLANE = nb * XW
        NPL = P1PAD + 1   # 18 data planes + 1 zero plane for the zero slots
        h1pool = ctx.enter_context(tc.tile_pool(name="h1p", bufs=1))
        h1p = h1pool.tile([128, NPL * PLANE + 64], FP8)
        h1v = h1p[:, 0:P1PAD * PLANE].rearrange(
            "p (y b x) -> p y b x", y=P1PAD, b=nb)
        # zero the pad borders, overscan columns, zero plane and tail
        nc.vector.memset(h1v[:, 0, :, :], 0.0)
        nc.vector.memset(h1v[:, P1PAD - 1, :, :], 0.0)
        nc.vector.memset(h1v[:, 1:P1PAD - 1, :, 0], 0.0)
        nc.vector.memset(h1v[:, 1:P1PAD - 1, :, 17:XW], 0.0)
        nc.vector.memset(h1p[:, P1PAD * PLANE:], 0.0)

        dwcpool = ctx.enter_context(tc.tile_pool(name="dwc", bufs=1))
        dwc = dwcpool.tile([128, nb * 256 + 16], FP8)
        # dwc layout: (b, y, x) with y,x in 16x16
        dwcv = dwc[:, 0:nb * 256].rearrange("p (b y x) -> p b y x",
                                            b=nb, y=P1)

        # DMA issue rotation, weighted toward engines with idle capacity
        # (only sync/gpsimd/scalar can trigger DMAs)
        dma_engines = [nc.sync, nc.gpsimd, nc.sync, nc.gpsimd,
                       nc.sync, nc.gpsimd, nc.sync, nc.scalar]

        DW_CHUNKS = [(0, 25), (25, 25), (50, 25), (75, 25), (100, 25),
                     (125, 3)]

        def dw_row(yo, b0, cnt, dps):
            """conv2 depthwise for output row yo (0..15), images b0..b0+cnt.

            Reads padded h1 planes yo..yo+2; plane yo+2 is written by conv1
            pooled row hc=yo+1 (or is the bottom pad for yo=15).  The rhs is
            a flat [p, 2, cnt*XW] DoubleRow AP; overscan columns (x>=16)
            are discarded at eviction.
            """
            pd = dps.tile([128, 25 * XW], F32, tag="dps")
            # last chunk drops 4 overscan cols so all reads stay in-bounds
            n = cnt * XW - (4 if b0 + cnt == nb else 0)
            npair = len(DW_PAIR_TAPS)
            for i, (ta, tb) in enumerate(DW_PAIR_TAPS):
                base = (yo + ta[0]) * PLANE
                boff = b0 * XW + ta[1]
                # [p, 2, n] with uniform plane delta, built from real slices
                # so the tile dependency tracker sees the exact regions
                rhs = h1p[:, base:base + 2 * PLANE].rearrange(
                    "p (two q) -> p two q", two=2)[:, :, boff:boff + n]
                nc.tensor.matmul(pd[:, 0:n], dwv_t[:, 2 * i:2 * i + 2, :],
                                 rhs, start=(i == 0), stop=(i == npair - 1),
                                 perf_mode=DR)
            nc.scalar.activation(
                dwcv[:, b0:b0 + cnt, yo, :],
                pd[:].rearrange("p (b x) -> p b x", x=XW)[:, 0:cnt, 0:P1],
                AF.Identity, bias=dwb_t[:])

        with tc.tile_pool(name="c1im", bufs=2) as impool, \
             tc.tile_pool(name="c1ev", bufs=2) as evpool, \
             tc.tile_pool(name="c1ps", bufs=1, space="PSUM") as pspool, \
             tc.tile_pool(name="dwps", bufs=2, space="PSUM") as dps:
            for hc in range(nh2):
                imt = impool.tile([82, 2 * rowlen], BF16, tag="im")
                # split across both HWDGE rings (sync + scalar) + SWDGE
                nc.sync.dma_start(imt[0:28, :],
                                  x81[hc * 82:hc * 82 + 28, :])
                nc.scalar.dma_start(imt[28:56, :],
                                    x81[hc * 82 + 28:hc * 82 + 56, :])
                nc.gpsimd.dma_start(imt[56:82, :],
                                    x81[hc * 82 + 56:hc * 82 + 82, :])
                imv = imt[:].rearrange("p (h b w2 dx) -> p h b w2 dx",
                                       h=2, b=nb, w2=HP // 2)
                for c in range(NCH):
                    qs = []
                    for dy in range(2):
                        for dx in range(2):
                            ps = pspool.tile([128, CB * 16], F32,
                                             tag=f"q{2 * dy + dx}")
                            nc.tensor.matmul(
                                ps[:], w1_t[:],
                                imv[:, dy, c * CB:(c + 1) * CB, 0:16, dx],
                                start=True, stop=True)
                            qs.append(ps)
                    out_sl = h1v[:, hc + 1, c * CB:(c + 1) * CB, 1:17]
                    # max-then-sign eviction; sign commutes with max, and
                    # TT may read at most one PSUM operand.  Alternate two
                    # chain types to balance ACT vs DVE load.
                    if (hc * NCH + c) % 8 < 3:
                        # X: ACT stages 2 raw copies, DVE maxes, ACT signs
                        e0 = evpool.tile([128, CB * 16], F32, tag="e0")
                        e2 = evpool.tile([128, CB * 16], F32, tag="e2")
                        nc.scalar.copy(e0[:], qs[0][:])
                        nc.scalar.copy(e2[:], qs[2][:])
                        t01 = evpool.tile([128, CB * 16], BF16, tag="t01")
                        t23 = evpool.tile([128, CB * 16], BF16, tag="t23")
                        nc.vector.tensor_max(t01[:], qs[1][:], e0[:])
                        nc.vector.tensor_max(t23[:], qs[3][:], e2[:])
                        tm = evpool.tile([128, CB * 16], BF16, tag="tm")
                        nc.vector.tensor_max(tm[:], t01[:], t23[:])
                        nc.scalar.activation(
                            out_sl, tm[:].rearrange("p (b x) -> p b x", b=CB),
                            AF.Sign)
                    else:
                        # Y: ACT signs all 4 quadrants, DVE bf16 max tree
                        ss = []
                        for q in range(4):
                            s = evpool.tile([128, CB * 16], BF16, tag=f"s{q}")
                            nc.scalar.activation(s[:], qs[q][:], AF.Sign)
                            ss.append(s)
                        y01 = evpool.tile([128, CB * 16], BF16, tag="t01")
                        y23 = evpool.tile([128, CB * 16], BF16, tag="t23")
                        nc.vector.tensor_max(y01[:], ss[0][:], ss[1][:])
                        nc.vector.tensor_max(y23[:], ss[2][:], ss[3][:])
                        nc.vector.tensor_max(
                            out_sl,
                            y01[:].rearrange("p (b x) -> p b x", b=CB),
                            y23[:].rearrange("p (b x) -> p b x", b=CB))
                    if DW_INTERLEAVE and hc >= 2:
                        # dw chunks interleave between conv1 chunks (2-row
                        # lag: dw(yo) reads planes yo..yo+3, all written)
                        for b0, cnt in DW_CHUNKS[c:c + (3 if c == 3 else 1)]:
                            dw_row(hc - 2, b0, cnt, dps)
            if DW_INTERLEAVE:
                for yo in (nh2 - 2, nh2 - 1):   # 2-row lag tail
                    for b0, cnt in DW_CHUNKS:
                        dw_row(yo, b0, cnt, dps)
            else:
                for yo in range(nh2):
                    for b0, cnt in DW_CHUNKS:
                        dw_row(yo, b0, cnt, dps)

        # ---- conv2 pointwise + pool2 -> h2t (fp8 +-1) ----
        dwc6 = dwc[:, 0:nb * 256].rearrange(
            "p (b y2 dy x2 dx) -> p b y2 dy x2 dx", b=nb, y2=P2, dy=2, x2=P2)
        with tc.tile_pool(name="pwev", bufs=2) as pevpool, \
             tc.tile_pool(name="pwps", bufs=2, space="PSUM") as pps:
            PJ = 8   # images per pw chunk
            for mt in range(2):
                for j in range(nb // PJ):
                    b0 = j * PJ
                    qs = []
                    for dy in range(2):
                        for dx in range(2):
                            ps = pps.tile([128, PJ * 64], F32,
                                          tag=f"pq{2 * dy + dx}")
                            rhs = dwc6[:, b0:b0 + PJ, :, dy, :, dx]
                            nc.tensor.matmul(
                                ps[:], pw_t[:, 128 * mt:128 * (mt + 1)],
                                rhs, start=True, stop=True)
                            qs.append(ps)
                    h2vv = h2t[mt][:].rearrange("p (s b) -> p b s", b=nb)
                    out_sl = h2vv[:, b0:b0 + PJ, :]
                    if True:
                        g0 = pevpool.tile([128, PJ * 64], F32, tag="g0")
                        g2 = pevpool.tile([128, PJ * 64], F32, tag="g2")
                        nc.scalar.copy(g0[:], qs[0][:])
                        nc.scalar.copy(g2[:], qs[2][:])
                        u01 = pevpool.tile([128, PJ * 64], BF16, tag="u01")
                        u23 = pevpool.tile([128, PJ * 64], BF16, tag="u23")
                        nc.vector.tensor_max(u01[:], qs[1][:], g0[:])
                        nc.vector.tensor_max(u23[:], qs[3][:], g2[:])
                        um = pevpool.tile([128, PJ * 64], BF16, tag="um")
                        nc.vector.tensor_max(um[:], u01[:], u23[:])
                        nc.scalar.activation(
                            out_sl, um[:].rearrange("p (b s) -> p b s", b=PJ),
                            AF.Sign, bias=b2_t[:, mt:mt + 1])
                    else:
                        ss = []
                        for q in range(4):
                            s = pevpool.tile([128, PJ * 64], BF16,
                                             tag=f"ps{q}")
                            nc.scalar.activation(s[:], qs[q][:], AF.Sign,
                                                 bias=b2_t[:, mt:mt + 1])
                            ss.append(s)
                        v01 = pevpool.tile([128, PJ * 64], BF16, tag="u01")
                        v23 = pevpool.tile([128, PJ * 64], BF16, tag="u23")
                        nc.vector.tensor_max(v01[:], ss[0][:], ss[1][:])
                        nc.vector.tensor_max(v23[:], ss[2][:], ss[3][:])
                        nc.vector.tensor_max(
                            out_sl,
                            v01[:].rearrange("p (b s) -> p b s", b=PJ),
                            v23[:].rearrange("p (b s) -> p b s", b=PJ))
        for mt in range(2):
            nc.sync.dma_start(h2_shard[mt], h2t[mt][:])

        import os as _os
        if _os.environ.get("BCN_DEBUG"):
            dbg_h1 = nc.dram_tensor("dbg_h1", [128, NPL * nb * XW + 64],
                                    FP8, kind="ExternalOutput").ap()
            nc.sync.dma_start(dbg_h1[:], h1p[:])
            dbg_dwc = nc.dram_tensor("dbg_dwc", [128, nb * 256 + 16], FP8,
                                     kind="ExternalOutput").ap()
            nc.sync.dma_start(dbg_dwc[:], dwc[:])
            dbg_h2 = nc.dram_tensor("dbg_h2", [2, 128, nb * 64], FP8,
                                    kind="ExternalOutput").ap()
            for mt in range(2):
                nc.sync.dma_start(dbg_h2[mt], h2t[mt][:])

    # ---- AllGather (raw phase) ----
    # No all_engine_barrier afterwards: all reads of h2_all in the fc phase
    # are issued from the gpsimd queue, which program-orders them after the
    # collective wait below; other engines (fc weight loads) may proceed
    # during the collective.
    if ncores > 1:
        with nc.Block() as blk, nc.semaphore("cc_sem") as cc_sem:
            @blk.gpsimd
            def _(gp):
                gp.collective_compute(
                    "AllGather", ALU.bypass,
                    replica_groups=[list(range(ncores))],
                    ins=[h2_shard], outs=[h2_all],
                ).then_inc(cc_sem)
                gp.wait_ge(cc_sem, 1)
        nc.all_engine_barrier()
    else:
        with nc.Block() as blk, nc.semaphore("cp_sem") as cp_sem:
            @blk.gpsimd
            def _(gp):
                gp.dma_start(h2_all[0], h2_shard[:]).then_inc(cp_sem, 16)
                gp.wait_ge(cp_sem, 16)
        nc.all_engine_barrier()

    # ---- fc1 + fc2 (Tile phase 2) ----
    with tile.TileContext(nc) as tc2, ExitStack() as ctx2:
        hgp = ctx2.enter_context(tc2.tile_pool(name="hg", bufs=1))
        wp = ctx2.enter_context(tc2.tile_pool(name="wfc", bufs=1))
        sp = ctx2.enter_context(tc2.tile_pool(name="fc1out", bufs=1))
        psp = ctx2.enter_context(tc2.tile_pool(name="fcps", bufs=1,
                                               space="PSUM"))
        p10 = ctx2.enter_context(tc2.tile_pool(name="fc2ps", bufs=2,
                                               space="PSUM"))
        yp = ctx2.enter_context(tc2.tile_pool(name="yout", bufs=1))
        dma_engines2 = [nc.sync, nc.gpsimd, nc.scalar]

        whi_sb = wp.tile([128, NKT * FPC], FP16, tag="whi", name="whi_sb")
        wlo_sb = wp.tile([128, NKT * FPC], FP16, tag="wlo", name="wlo_sb")
        whi_v = whi_sb[:].rearrange("p (k c) -> p k c", k=NKT)
        wlo_v = wlo_sb[:].rearrange("p (k c) -> p k c", k=NKT)

        hg = {}
        for ct in range(2):
            for bcc in range(nbc):
                t = hgp.tile([128, sh_per_bc * nb * 64], FP8,
                             tag=f"hg{ct}{bcc}", name=f"hg{ct}{bcc}")
                hg[(ct, bcc)] = t

        # weight loads (no collective dependence) overlap the AllGather;
        # hg loads MUST be issued from gpsimd (queued after the cc wait)
        HKT = NKT * FPC // 2
        nc.sync.dma_start(whi_sb[:, 0:HKT], whi[:, 0:HKT])
        nc.scalar.dma_start(wlo_sb[:, 0:HKT], wlo[:, 0:HKT])
        nc.sync.dma_start(whi_sb[:, HKT:], whi[:, HKT:])
        nc.scalar.dma_start(wlo_sb[:, HKT:], wlo[:, HKT:])
        SHB = nb * 64
        for bcc in range(nbc):
            for ct in range(2):
                src = AP(h2_all.tensor,
                         (bcc * sh_per_bc * 2 + ct) * 128 * SHB,
                         [[SHB, 128], [2 * 128 * SHB, sh_per_bc], [1, SHB]])
                nc.gpsimd.dma_start(hg[(ct, bcc)][:], src)

        s1 = sp.tile([128, nball], FP8)
        psf = [psp.tile([128, bc_n], F32, tag=f"psf{bcc}", name=f"psf{bcc}")
               for bcc in range(nbc)]
        f2hi_t = sp.tile([128, 10], FP16)
        nc.sync.dma_start(f2hi_t[:], f2hi[:])
        f2lo_t = sp.tile([128, 10], FP16)
        nc.sync.dma_start(f2lo_t[:], f2lo[:])

        yt = yp.tile([10, nball], F32)
        for bcc in range(nbc):
            for kt in range(NKT):
                ct, s0 = kt // 64, kt % 64
                rhs = hg[(ct, bcc)][:].rearrange(
                    "p (s x b) -> p s x b",
                    s=sh_per_bc, b=nb)[:, :, s0, :]
                nc.tensor.matmul(psf[bcc][:], whi_v[:, kt, :], rhs,
                                 start=(kt == 0), stop=False)
                nc.tensor.matmul(psf[bcc][:], wlo_v[:, kt, :], rhs,
                                 start=False, stop=(kt == NKT - 1))
            nc.scalar.activation(s1[:, bcc * bc_n:(bcc + 1) * bc_n],
                                 psf[bcc][:], AF.Sign)
            ps10 = p10.tile([10, bc_n], F32, tag="ps10")
            nc.tensor.matmul(ps10[:], f2hi_t[:],
                             s1[:, bcc * bc_n:(bcc + 1) * bc_n],
                             start=True, stop=False)
            nc.tensor.matmul(ps10[:], f2lo_t[:],
                             s1[:, bcc * bc_n:(bcc + 1) * bc_n],
                             start=False, stop=True)
            nc.scalar.copy(yt[:, bcc * bc_n:(bcc + 1) * bc_n], ps10[:])
        nc.sync.dma_start(y_out[:], yt[:])

    nc.compile()
    return nc


_CACHE = {}


def _get_program(ncores=NCORES, nb=B):
    key = (ncores, nb)
    if key not in _CACHE:
        _CACHE[key] = build_program(ncores, nb)
    return _CACHE[key]


def kernel(**inputs):
    per_core = _host_prep(**inputs)
    nc = _get_program()
    res = run_bass_kernel_spmd(nc, per_core, core_ids=list(range(NCORES)))
    fc2_b = np.asarray(inputs["fc2_b"], np.float32)
    y = np.zeros((10, NB_ALL), np.float32)
    for n in range(NCORES):
        y += res.results[n]["y"]
    return (y.T + fc2_b[None, :]).astype(np.float32)
